# revision 13
# baseline (speedup 1.0000x reference)
"""Multi-head self-attention Trainium2 kernel (8-core tensor-parallel).

Problem: B=2, N=2048, C=1024, H=16 heads, HD=64.

Sharding (v3 — minimizes per-call host<->device traffic AND HW time):
  - Inputs are token-sharded: core c receives x^T columns for 256 batch-0
    tokens and 256 batch-1 tokens (bf16, 0.5 MB each) plus its 2-head qkv
    weight slice and 128 rows of w_proj (bf16).  Two on-device AllGathers
    rebuild the full x^T (batch 0 first, so compute starts after a
    half-size gather); a third AllGather rebuilds w_proj during attention.
  - Attention runs head-parallel (2 heads per core, full [N, N] scores,
    softmax denominators produced as row 64 of the attn@v accumulation via
    a constant-ones column appended to v).
  - Two on-device AllToAlls re-shard the attention output from head-major
    to token-major (batch 0's runs during batch-1 attention), so every
    core computes the FULL output projection for its own 512 tokens and
    writes a [512, C] bf16 slice.  The host concatenates the 8 slices
    (rows = 256 batch-0 tokens then 256 batch-1 tokens per core) and adds
    b_proj.

All matmuls run in bf16 (fp32 PSUM accumulation): simulated end-to-end
max-rel error ~6e-3 vs fp64 reference (gate 2e-2).
"""

import numpy as np

B, N, C = 2, 2048, 1024
H = 16
HD = C // H  # 64
SCALE = HD ** -0.5
T = B * N  # 4096 tokens
NCORES = 8
HPC = H // NCORES  # 2 heads per core
TS = T // NCORES   # 512 tokens per core (256 per batch)
HS = TS // B       # 256-token half-shards

_CACHE = {}


def _build_program():
    import concourse.bass as bass
    import concourse.mybir as mybir
    import concourse.tile as tile
    from concourse import bacc

    f32 = mybir.dt.float32
    bf16 = mybir.dt.bfloat16
    Exp = mybir.ActivationFunctionType.Exp
    Mult = mybir.AluOpType.mult

    nc = bacc.Bacc("TRN2", target_bir_lowering=False, debug=False,
                   num_devices=NCORES)

    xa_d = nc.dram_tensor("xTsA", [C, HS], bf16, kind="ExternalInput")
    xb_d = nc.dram_tensor("xTsB", [C, HS], bf16, kind="ExternalInput")
    wq_d = nc.dram_tensor("w_loc", [C, 3 * HPC * HD], bf16,
                          kind="ExternalInput")
    bq_d = nc.dram_tensor("b_loc", [128, 3], f32, kind="ExternalInput")
    w2s_d = nc.dram_tensor("w2s", [HPC * HD, C], bf16, kind="ExternalInput")
    id_d = nc.dram_tensor("ident", [128, 128], bf16, kind="ExternalInput")
    sel2_d = nc.dram_tensor("sel2", [128, 128], f32, kind="ExternalInput")
    out_d = nc.dram_tensor("out_s", [TS, C], bf16, kind="ExternalOutput")

    CC = C // 128            # 8 contraction chunks over C
    NF = 3 * HPC * HD // 128  # 3 feature chunks (q, k, v)
    NTB = T // TS            # 8 token blocks of 512
    NKC = N // 128           # 16 key chunks per batch
    NQB = N // 512           # 4 query blocks of 512 per batch
    NTC = T // 128           # 32 token chunks
    RG = [list(range(NCORES))]

    with tile.TileContext(nc) as tc:
        with tc.tile_pool(name="persist", bufs=1) as persist, \
             tc.tile_pool(name="dram", bufs=1, space="DRAM") as dram, \
             tc.tile_pool(name="xt", bufs=3, space="SBUF") as xt_pool, \
             tc.tile_pool(name="exp", bufs=4) as exp_pool, \
             tc.tile_pool(name="small", bufs=4) as small_pool, \
             tc.tile_pool(name="ob", bufs=3) as out_pool, \
             tc.tile_pool(name="ps", bufs=2, space="PSUM") as psum_s, \
             tc.tile_pool(name="pq", bufs=2, space="PSUM") as psum_q, \
             tc.tile_pool(name="po", bufs=2, space="PSUM") as psum_o:

            # ---- persistent SBUF tensors ----
            w_sb = persist.tile([128, CC, 3 * HPC * HD], bf16, tag="w_sb")
            b_sb = persist.tile([128, 3], f32, tag="b_sb")
            ident = persist.tile([128, 128], bf16, tag="ident")
            sel2 = persist.tile([128, 128], f32, tag="sel2")
            # denominator staging rows: head h's sums land at partition 64*h;
            # remaining partitions are zeroed once so the broadcast matmul
            # (sel2 has zero columns there) never multiplies garbage.
            s2 = persist.tile([128, 512], f32, tag="s2")
            nc.vector.memset(s2[:], 0.0)
            qT = persist.tile([128, T], bf16, tag="qT")
            kT = persist.tile([128, T], bf16, tag="kT")
            vT = persist.tile([128, T], bf16, tag="vT")
            # natural-layout v per token-chunk: [vA(64) | 1 | vB(64) | 1]
            v_nat = persist.tile([128, NTC, 130], bf16, tag="v_nat")
            ohT = persist.tile([128, T], bf16, tag="ohT")
            w2_sb = persist.tile([128, CC, C], bf16, tag="w2_sb")
            # token-major attention output: [dims-of-rank r, batch s, 256]
            oh_all = persist.tile([128, NTB, B, HS], bf16, tag="oh_all")

            # ---- DRAM bounce buffers for collectives ----
            xga_in = dram.tile([C, HS], bf16)
            xga = dram.tile([C * NCORES, HS], bf16, addr_space="Shared")
            xgb_in = dram.tile([C, HS], bf16)
            xgb = dram.tile([C * NCORES, HS], bf16, addr_space="Shared")
            w2g_in = dram.tile([HPC * HD, C], bf16)
            w2g = dram.tile([HPC * HD * NCORES, C], bf16, addr_space="Shared")
            a2aA_in = dram.tile([128 * NCORES, HS], bf16)
            a2aA_out = dram.tile([128 * NCORES, HS], bf16)
            a2aB_in = dram.tile([128 * NCORES, HS], bf16)
            a2aB_out = dram.tile([128 * NCORES, HS], bf16)

            # ---- x gathers (batch 0 first; compute starts on its arrival)
            nc.sync.dma_start(out=xga_in[:], in_=xa_d[:])
            nc.gpsimd.collective_compute(
                "AllGather", mybir.AluOpType.bypass, replica_groups=RG,
                ins=[xga_in[:]], outs=[xga[:]])
            nc.sync.dma_start(out=xgb_in[:], in_=xb_d[:])
            nc.gpsimd.collective_compute(
                "AllGather", mybir.AluOpType.bypass, replica_groups=RG,
                ins=[xgb_in[:]], outs=[xgb[:]])
            nc.sync.dma_start(out=w2g_in[:], in_=w2s_d[:])

            # ---- constants ----
            nc.sync.dma_start(
                out=w_sb[:],
                in_=wq_d[:].rearrange("(cc p) f -> p cc f", p=128))
            nc.sync.dma_start(out=b_sb[:], in_=bq_d[:])
            nc.sync.dma_start(out=ident[:], in_=id_d[:])
            nc.sync.dma_start(out=sel2[:], in_=sel2_d[:])
            # ones columns for the softmax-denominator rows
            nc.vector.memset(v_nat[:, :, 64:65], 1.0)
            nc.vector.memset(v_nat[:, :, 129:130], 1.0)

            qkvT = [qT, kT, vT]

            def v_nat_copy(pt, tcg):
                # strided copy: pt cols [0:64],[64:128] -> v_nat cols
                # [0:64],[65:129] (skipping the ones column)
                src = pt[:, 0:128]
                dst = v_nat[:, tcg, 0:129]
                nc.vector.tensor_copy(
                    bass.AP(tensor=dst.tensor, offset=dst.offset,
                            ap=[list(dst.ap[0]), [65, 2], [1, 64]]),
                    bass.AP(tensor=src.tensor, offset=src.offset,
                            ap=[list(src.ap[0]), [64, 2], [1, 64]]))

            # ---- phase 1: qkv^T = w_loc^T @ x^T per 512-token block ----
            def emit_qkv(tb):
                # block tb covers global tokens [tb*512, (tb+1)*512) of
                # batch tb//4, stored in gathered shards 2*(tb%4), +1
                xg = xga if tb < NTB // B else xgb
                j0 = (tb % (NTB // B)) * 2
                xt = xt_pool.tile([128, CC, 512], bf16, tag="xt",
                                  name=f"xt_{tb}")
                for s in range(2):
                    src = xg[(j0 + s) * C:(j0 + s + 1) * C, :]
                    nc.sync.dma_start(
                        out=xt[:, :, s * HS:(s + 1) * HS],
                        in_=src.rearrange("(cc p) t -> p cc t", p=128))
                xts = [xt[:, ci, :] for ci in range(CC)]
                for fc in range(NF):
                    ps = psum_q.tile([128, 512], f32, tag="q",
                                     name=f"ps1_{tb}_{fc}")
                    for ci in range(CC):
                        nc.tensor.matmul(
                            ps[:],
                            w_sb[:, ci, fc * 128:(fc + 1) * 128],
                            xts[ci],
                            start=(ci == 0), stop=(ci == CC - 1))
                    nc.vector.tensor_scalar_add(
                        qkvT[fc][:, tb * 512:(tb + 1) * 512],
                        ps[:], b_sb[:, fc:fc + 1])
                # transpose this block's v^T chunks to natural layout
                for tcq in range(4):
                    tcg = tb * 4 + tcq
                    pt = psum_q.tile([128, 1024], bf16, tag="q",
                                     name=f"pt_{tcg}")
                    sl = slice(tcg * 128, (tcg + 1) * 128)
                    nc.tensor.transpose(pt[:, 0:128], vT[:, sl], ident[:])
                    v_nat_copy(pt, tcg)

            # ---- phase 2: attention per (batch, query block) ----
            def emit_attention(b, qb):
                qsl = slice(b * N + qb * 512, b * N + (qb + 1) * 512)
                po = [psum_o.tile([128, 512], f32, tag="po",
                                  name=f"po_{b}_{qb}_{h}")
                      for h in range(HPC)]
                for kcg in range(NKC // 2):
                    pss = {}
                    for h in range(HPC):
                        pss[h] = psum_s.tile([128, 1024], f32, tag="s",
                                             name=f"ps2_{b}_{qb}_{kcg}_{h}")
                    # alternate heads between consecutive matmuls so
                    # LDWEIGHTS (row groups 0-1 vs 2-3) overlaps the
                    # in-flight matmul of the other head
                    for kc2 in range(2):
                        kc = kcg * 2 + kc2
                        ksl = slice(b * N + kc * 128, b * N + (kc + 1) * 128)
                        for h in range(HPC):
                            hsl = slice(h * 64, (h + 1) * 64)
                            nc.tensor.matmul(
                                pss[h][:, kc2 * 512:(kc2 + 1) * 512],
                                kT[hsl, ksl], qT[hsl, qsl],
                                start=True, stop=True)
                    exs = {}
                    for h in range(HPC):
                        ex = exp_pool.tile([128, 1024], bf16, tag="ex",
                                           name=f"ex_{b}_{qb}_{kcg}_{h}")
                        nc.scalar.activation(ex[:], pss[h][:], Exp,
                                             scale=float(SCALE))
                        exs[h] = ex
                    for kc2 in range(2):
                        kc = kcg * 2 + kc2
                        tcg = b * NKC + kc
                        for h in range(HPC):
                            nc.tensor.matmul(
                                po[h][0:65, :],
                                v_nat[:, tcg, h * 65:(h + 1) * 65],
                                exs[h][:, kc2 * 512:(kc2 + 1) * 512],
                                start=(kc == 0),
                                stop=(kc == NKC - 1))
                # normalization: reciprocal of denominator row, broadcast
                # to both heads' 64 partitions via one PE outer product
                for h in range(HPC):
                    nc.vector.tensor_copy(s2[h * 64:h * 64 + 1, :],
                                          po[h][64:65, :])
                prd = psum_q.tile([128, 512], f32, tag="q",
                                  name=f"prd_{b}_{qb}")
                nc.tensor.matmul(prd[:], sel2[:], s2[:],
                                 start=True, stop=True)
                rcp = small_pool.tile([128, 512], f32, tag="rb",
                                      name=f"rcp_{b}_{qb}")
                nc.vector.reciprocal_approx_fast(out=rcp[:], in_=prd[:])
                for h in range(HPC):
                    nc.vector.tensor_tensor(
                        ohT[h * 64:(h + 1) * 64, qsl],
                        po[h][0:64, :], rcp[h * 64:(h + 1) * 64, :], Mult)

            # ---- phase 3: AllToAll + full projection for own tokens ----
            def emit_a2a(s):
                # re-shard batch s: shard j = ohT columns of core j's
                # 256 batch-s tokens
                a_in = a2aA_in if s == 0 else a2aB_in
                a_out = a2aA_out if s == 0 else a2aB_out
                nc.sync.dma_start(
                    out=a_in[:].rearrange("(blk p) t -> p blk t", p=128),
                    in_=ohT[:, s * N:(s + 1) * N].rearrange(
                        "p (blk t) -> p blk t", blk=NTB))
                nc.gpsimd.collective_compute(
                    "AllToAll", mybir.AluOpType.bypass, replica_groups=RG,
                    ins=[a_in[:]], outs=[a_out[:]])
                nc.sync.dma_start(
                    out=oh_all[:, :, s, :],
                    in_=a_out[:].rearrange("(blk p) t -> p blk t", p=128))

            def emit_proj(tcq):
                # own tokens tcq*128..: chunks 0-1 are batch-0 halves,
                # 2-3 batch-1 (oh_all free layout is [blk, s, 256])
                pp = psum_s.tile([128, 1024], f32, tag="s",
                                 name=f"pp_{tcq}")
                oh_flat = oh_all[:].rearrange("p blk s t -> p blk (s t)")
                for jh in range(C // 512):
                    for r in range(NCORES):
                        nc.tensor.matmul(
                            pp[:, jh * 512:(jh + 1) * 512],
                            oh_flat[:, r, tcq * 128:(tcq + 1) * 128],
                            w2_sb[:, r, jh * 512:(jh + 1) * 512],
                            start=(r == 0), stop=(r == NCORES - 1))
                ob = out_pool.tile([128, 1024], bf16, tag="ob",
                                   name=f"ob_{tcq}")
                nc.vector.tensor_copy(ob[:], pp[:])
                nc.sync.dma_start(
                    out=out_d[tcq * 128:(tcq + 1) * 128, :],
                    in_=ob[:])

            # ---- emission order: batch-0 qkv; batch-0 attention with
            # batch-1 qkv interleaved; w2 gather + batch-0 AllToAll and
            # half the projection during batch-1 attention ----
            for tb in range(NTB // B):
                emit_qkv(tb)
            for qb in range(NQB):
                emit_attention(0, qb)
                emit_qkv(NTB // B + qb)
            # w_proj gather: triggered here so ncfw runs it while the
            # engines chew on attention; needed only by the projection
            nc.gpsimd.collective_compute(
                "AllGather", mybir.AluOpType.bypass, replica_groups=RG,
                ins=[w2g_in[:]], outs=[w2g[:]])
            nc.sync.dma_start(
                out=w2_sb[:],
                in_=w2g[:].rearrange("(cc p) f -> p cc f", p=128))
            emit_a2a(0)
            for qb in range(NQB):
                emit_attention(1, qb)
                if qb >= 2:
                    emit_proj(qb - 2)  # batch-0 projection chunks 0-1
            emit_a2a(1)
            for tcq in range(2, 4):
                emit_proj(tcq)

    nc.compile()
    return nc


def get_program():
    if "nc" not in _CACHE:
        _CACHE["nc"] = _build_program()
    return _CACHE["nc"]


def build_null_program():
    """Tiny kernel for calibrating per-dispatch overhead in test harnesses."""
    import concourse.mybir as mybir
    import concourse.tile as tile
    from concourse import bacc

    f32 = mybir.dt.float32
    nc = bacc.Bacc("TRN2", target_bir_lowering=False, debug=False,
                   num_devices=NCORES)
    x_in = nc.dram_tensor("x", [128, 128], f32, kind="ExternalInput")
    y_out = nc.dram_tensor("y", [128, 128], f32, kind="ExternalOutput")
    with tile.TileContext(nc) as tc:
        with tc.tile_pool(name="p", bufs=1) as pool:
            t = pool.tile([128, 128], f32)
            nc.sync.dma_start(out=t[:], in_=x_in[:])
            nc.sync.dma_start(out=y_out[:], in_=t[:])
    nc.compile()
    x = np.zeros((128, 128), dtype=np.float32)
    return nc, [{"x": x} for _ in range(NCORES)]


def make_in_maps(x, w_qkv, b_qkv, w_proj):
    """Host-side sharding: per-core input dicts (bf16 weights/activations).

    Core c owns batch-0 tokens [c*256,(c+1)*256) and the same range of
    batch 1."""
    import ml_dtypes
    bf16 = ml_dtypes.bfloat16

    xT = np.ascontiguousarray(x.reshape(T, C).T.astype(bf16))
    ident = np.eye(128, dtype=bf16)
    sel2 = np.zeros((128, 128), dtype=np.float32)
    for h in range(HPC):
        sel2[h * 64, h * 64:(h + 1) * 64] = 1.0
    in_maps = []
    for core in range(NCORES):
        heads = [core * HPC + h for h in range(HPC)]
        cols = []
        for s in range(3):  # q, k, v groups
            for h in heads:
                cols.append(np.arange(s * C + h * HD, s * C + (h + 1) * HD))
        cols = np.concatenate(cols)
        w_loc = np.ascontiguousarray(w_qkv[:, cols].astype(bf16))
        b_loc = np.ascontiguousarray(
            b_qkv[cols].reshape(3, HPC * HD).T).astype(np.float32)
        rows = np.concatenate(
            [np.arange(h * HD, (h + 1) * HD) for h in heads])
        w2s = np.ascontiguousarray(w_proj[rows, :].astype(bf16))
        in_maps.append({
            "xTsA": np.ascontiguousarray(
                xT[:, core * HS:(core + 1) * HS]),
            "xTsB": np.ascontiguousarray(
                xT[:, N + core * HS:N + (core + 1) * HS]),
            "w_loc": w_loc,
            "b_loc": b_loc,
            "w2s": w2s,
            "ident": ident,
            "sel2": sel2,
        })
    return in_maps


def combine_results(results, b_proj):
    """Host-side unshard: interleave the 8 token slices, add bias."""
    out = np.empty((B, N, C), dtype=np.float32)
    for core, res in enumerate(results):
        sl = np.asarray(res["out_s"], dtype=np.float32)
        out[0, core * HS:(core + 1) * HS] = sl[0:HS]
        out[1, core * HS:(core + 1) * HS] = sl[HS:2 * HS]
    out += b_proj.astype(np.float32)[None, None, :]
    return out


def kernel(x, w_qkv, b_qkv, w_proj, b_proj):
    from concourse.bass_utils import run_bass_kernel_spmd

    x = np.asarray(x, dtype=np.float32)
    w_qkv = np.asarray(w_qkv, dtype=np.float32)
    b_qkv = np.asarray(b_qkv, dtype=np.float32)
    w_proj = np.asarray(w_proj, dtype=np.float32)
    b_proj = np.asarray(b_proj, dtype=np.float32)

    nc = get_program()
    in_maps = make_in_maps(x, w_qkv, b_qkv, w_proj)
    res = run_bass_kernel_spmd(nc, in_maps, list(range(NCORES)))
    return combine_results(res.results, b_proj)


# revision 28
# speedup vs baseline: 3.4756x; 3.4756x over previous
"""Multi-head self-attention Trainium2 kernel (8-core tensor-parallel).

Problem: B=2, N=2048, C=1024, H=16 heads, HD=64.

Sharding (v3 — minimizes per-call host<->device traffic AND HW time):
  - Inputs are token-sharded: core c receives x^T columns for 256 batch-0
    tokens and 256 batch-1 tokens (bf16, 0.5 MB each) plus its 2-head qkv
    weight slice and 128 rows of w_proj (bf16).  Two on-device AllGathers
    rebuild the full x^T (batch 0 first, so compute starts after a
    half-size gather); a third AllGather rebuilds w_proj during attention.
  - Attention runs head-parallel (2 heads per core, full [N, N] scores,
    softmax denominators produced as row 64 of the attn@v accumulation via
    a constant-ones column appended to v).
  - Two on-device AllToAlls re-shard the attention output from head-major
    to token-major (batch 0's runs during batch-1 attention), so every
    core computes the FULL output projection for its own 512 tokens and
    writes a [512, C] bf16 slice.  The host concatenates the 8 slices
    (rows = 256 batch-0 tokens then 256 batch-1 tokens per core) and adds
    b_proj.

All matmuls run in bf16 (fp32 PSUM accumulation): simulated end-to-end
max-rel error ~6e-3 vs fp64 reference (gate 2e-2).
"""

import numpy as np

B, N, C = 2, 2048, 1024
H = 16
HD = C // H  # 64
SCALE = HD ** -0.5
T = B * N  # 4096 tokens
NCORES = 8
HPC = H // NCORES  # 2 heads per core
TS = T // NCORES   # 512 tokens per core (256 per batch)
HS = TS // B       # 256-token half-shards
CC_H = C // 128    # 8 contraction chunks (host-side mirror of CC)

_CACHE = {}


def _build_program():
    import concourse.bass as bass
    import concourse.mybir as mybir
    import concourse.tile as tile
    from concourse import bacc

    f32 = mybir.dt.float32
    bf16 = mybir.dt.bfloat16
    Exp = mybir.ActivationFunctionType.Exp
    Mult = mybir.AluOpType.mult

    nc = bacc.Bacc("TRN2", target_bir_lowering=False, debug=False,
                   num_devices=NCORES)

    # x half-shards, host-permuted to [partition, cc*256+t] so the gathered
    # blocks DMA into SBUF with contiguous 2 KiB per-partition reads
    xa_d = nc.dram_tensor("xTsA", [128, CC_H * HS], bf16,
                          kind="ExternalInput")
    xb_d = nc.dram_tensor("xTsB", [128, CC_H * HS], bf16,
                          kind="ExternalInput")
    wq_d = nc.dram_tensor("w_loc", [C, 3 * HPC * HD], bf16,
                          kind="ExternalInput")
    bq_d = nc.dram_tensor("b_loc", [128, 3], f32, kind="ExternalInput")
    w2s_d = nc.dram_tensor("w2s", [HPC * HD, C], bf16, kind="ExternalInput")
    id_d = nc.dram_tensor("ident", [128, 128], bf16, kind="ExternalInput")
    out_d = nc.dram_tensor("out_s", [TS, C], bf16, kind="ExternalOutput")

    CC = C // 128            # 8 contraction chunks over C
    NF = 3 * HPC * HD // 128  # 3 feature chunks (q, k, v)
    NTB = T // TS            # 8 token blocks of 512
    NKC = N // 128           # 16 key chunks per batch
    NQB = N // 512           # 4 query blocks of 512 per batch
    NTC = T // 128           # 32 token chunks
    RG = [list(range(NCORES))]

    with tile.TileContext(nc) as tc:
        with tc.tile_pool(name="persist", bufs=1) as persist, \
             tc.tile_pool(name="dram", bufs=1, space="DRAM") as dram, \
             tc.tile_pool(name="xt", bufs=3, space="SBUF") as xt_pool, \
             tc.tile_pool(name="exp", bufs=4) as exp_pool, \
             tc.tile_pool(name="small", bufs=4) as small_pool, \
             tc.tile_pool(name="ob", bufs=3) as out_pool, \
             tc.tile_pool(name="ps", bufs=2, space="PSUM") as psum_s, \
             tc.tile_pool(name="pq", bufs=2, space="PSUM") as psum_q, \
             tc.tile_pool(name="po", bufs=2, space="PSUM") as psum_o:

            # ---- persistent SBUF tensors ----
            w_sb = persist.tile([128, CC, 3 * HPC * HD], bf16, tag="w_sb")
            b_sb = persist.tile([128, 3], f32, tag="b_sb")
            ident = persist.tile([128, 128], bf16, tag="ident")
            qT = persist.tile([128, T], bf16, tag="qT")
            kT = persist.tile([128, T], bf16, tag="kT")
            vT = persist.tile([128, T], bf16, tag="vT")
            # natural-layout v per token-chunk: [vA(64) | 1 | vB(64) | 1]
            v_nat = persist.tile([128, NTC, 130], bf16, tag="v_nat")
            ohT = persist.tile([128, T], bf16, tag="ohT")
            w2_sb = persist.tile([128, CC, C], bf16, tag="w2_sb")
            # token-major attention output: [dims-of-rank r, batch s, 256]
            oh_all = persist.tile([128, NTB, B, HS], bf16, tag="oh_all")

            # ---- DRAM bounce buffers for collectives ----
            xga_in = dram.tile([128, CC * HS], bf16)
            xga = dram.tile([128 * NCORES, CC * HS], bf16,
                            addr_space="Shared")
            xgb_in = dram.tile([128, CC * HS], bf16)
            xgb = dram.tile([128 * NCORES, CC * HS], bf16,
                            addr_space="Shared")
            w2g_in = dram.tile([HPC * HD, C], bf16)
            w2g = dram.tile([HPC * HD * NCORES, C], bf16, addr_space="Shared")
            a2aA_in = dram.tile([128 * NCORES, HS], bf16)
            a2aA_out = dram.tile([128 * NCORES, HS], bf16)
            a2aB_in = dram.tile([128 * NCORES, HS], bf16)
            a2aB_out = dram.tile([128 * NCORES, HS], bf16)

            # ---- x gathers (batch 0 first; compute starts on its arrival)
            nc.sync.dma_start(out=xga_in[:], in_=xa_d[:])
            nc.gpsimd.collective_compute(
                "AllGather", mybir.AluOpType.bypass, replica_groups=RG,
                ins=[xga_in[:]], outs=[xga[:]])
            nc.sync.dma_start(out=xgb_in[:], in_=xb_d[:])
            nc.gpsimd.collective_compute(
                "AllGather", mybir.AluOpType.bypass, replica_groups=RG,
                ins=[xgb_in[:]], outs=[xgb[:]])
            nc.sync.dma_start(out=w2g_in[:], in_=w2s_d[:])

            # ---- constants ----
            nc.sync.dma_start(
                out=w_sb[:],
                in_=wq_d[:].rearrange("(cc p) f -> p cc f", p=128))
            nc.sync.dma_start(out=b_sb[:], in_=bq_d[:])
            nc.sync.dma_start(out=ident[:], in_=id_d[:])
            # ones columns for the softmax-denominator rows
            nc.vector.memset(v_nat[:, :, 64:65], 1.0)
            nc.vector.memset(v_nat[:, :, 129:130], 1.0)

            qkvT = [qT, kT, vT]

            def v_nat_copy(pt, tcg):
                # strided copy: pt cols [0:64],[64:128] -> v_nat cols
                # [0:64],[65:129] (skipping the ones column)
                src = pt[:, 0:128]
                dst = v_nat[:, tcg, 0:129]
                nc.vector.tensor_copy(
                    bass.AP(tensor=dst.tensor, offset=dst.offset,
                            ap=[list(dst.ap[0]), [65, 2], [1, 64]]),
                    bass.AP(tensor=src.tensor, offset=src.offset,
                            ap=[list(src.ap[0]), [64, 2], [1, 64]]))

            # ---- phase 1: qkv^T = w_loc^T @ x^T per 512-token block ----
            def emit_qkv(tb):
                # block tb covers global tokens [tb*512, (tb+1)*512) of
                # batch tb//4, stored in gathered shards 2*(tb%4), +1
                xg = xga if tb < NTB // B else xgb
                j0 = (tb % (NTB // B)) * 2
                xt = xt_pool.tile([128, CC, 512], bf16, tag="xt",
                                  name=f"xt_{tb}")
                for s in range(2):
                    src = xg[(j0 + s) * 128:(j0 + s + 1) * 128, :]
                    nc.sync.dma_start(
                        out=xt[:, :, s * HS:(s + 1) * HS],
                        in_=src.rearrange("p (cc t) -> p cc t", cc=CC))
                xts = [xt[:, ci, :] for ci in range(CC)]
                for fc in range(NF):
                    ps = psum_q.tile([128, 512], f32, tag="q",
                                     name=f"ps1_{tb}_{fc}")
                    for ci in range(CC):
                        nc.tensor.matmul(
                            ps[:],
                            w_sb[:, ci, fc * 128:(fc + 1) * 128],
                            xts[ci],
                            start=(ci == 0), stop=(ci == CC - 1))
                    nc.vector.tensor_scalar_add(
                        qkvT[fc][:, tb * 512:(tb + 1) * 512],
                        ps[:], b_sb[:, fc:fc + 1])
                # transpose this block's v^T chunks to natural layout
                for tcq in range(4):
                    tcg = tb * 4 + tcq
                    pt = psum_q.tile([128, 1024], bf16, tag="q",
                                     name=f"pt_{tcg}")
                    sl = slice(tcg * 128, (tcg + 1) * 128)
                    nc.tensor.transpose(pt[:, 0:128], vT[:, sl], ident[:])
                    v_nat_copy(pt, tcg)

            # ---- phase 2: attention per (batch, query block) ----
            def emit_attention(b, qb):
                qsl = slice(b * N + qb * 512, b * N + (qb + 1) * 512)
                po = [psum_o.tile([128, 512], f32, tag="po",
                                  name=f"po_{b}_{qb}_{h}")
                      for h in range(HPC)]
                for kcg in range(NKC // 2):
                    pss = {}
                    for h in range(HPC):
                        pss[h] = psum_s.tile([128, 1024], f32, tag="s",
                                             name=f"ps2_{b}_{qb}_{kcg}_{h}")
                    # the two heads occupy disjoint PE row halves
                    # (contraction dim 64): explicit tile_position packs
                    # them as concurrent row-group tiles
                    for kc2 in range(2):
                        kc = kcg * 2 + kc2
                        ksl = slice(b * N + kc * 128, b * N + (kc + 1) * 128)
                        for h in range(HPC):
                            hsl = slice(h * 64, (h + 1) * 64)
                            nc.tensor.matmul(
                                pss[h][:, kc2 * 512:(kc2 + 1) * 512],
                                kT[hsl, ksl], qT[hsl, qsl],
                                start=True, stop=True,
                                tile_position=(h * 64, 0))
                    exs = {}
                    for h in range(HPC):
                        ex = exp_pool.tile([128, 1024], bf16, tag="ex",
                                           name=f"ex_{b}_{qb}_{kcg}_{h}")
                        nc.scalar.activation(ex[:], pss[h][:], Exp,
                                             scale=float(SCALE))
                        exs[h] = ex
                    for kc2 in range(2):
                        kc = kcg * 2 + kc2
                        tcg = b * NKC + kc
                        for h in range(HPC):
                            nc.tensor.matmul(
                                po[h][0:65, :],
                                v_nat[:, tcg, h * 65:(h + 1) * 65],
                                exs[h][:, kc2 * 512:(kc2 + 1) * 512],
                                start=(kc == 0),
                                stop=(kc == NKC - 1))
                # normalization: denominator rows to SBUF, reciprocal of
                # the [1, 512] rows, partition-broadcast on the idle
                # GpSimd engine (keeps the PE instruction queue clear).
                # partition_broadcast only writes from base partition 0,
                # so each head gets its own [64, 512] tile.
                for h in range(HPC):
                    sh = small_pool.tile([1, 512], f32, tag=f"sh{h}",
                                         name=f"sh_{b}_{qb}_{h}")
                    shr = small_pool.tile([1, 512], f32, tag=f"shr{h}",
                                          name=f"shr_{b}_{qb}_{h}")
                    rcp = small_pool.tile([64, 512], f32, tag=f"rb{h}",
                                          name=f"rcp_{b}_{qb}_{h}")
                    nc.vector.tensor_copy(sh[:], po[h][64:65, :])
                    nc.vector.reciprocal_approx_fast(out=shr[:], in_=sh[:])
                    nc.gpsimd.partition_broadcast(rcp[:], shr[:])
                    nc.vector.tensor_tensor(
                        ohT[h * 64:(h + 1) * 64, qsl],
                        po[h][0:64, :], rcp[:], Mult)

            # ---- phase 3: AllToAll + full projection for own tokens ----
            def emit_a2a(s):
                # re-shard batch s: shard j = ohT columns of core j's
                # 256 batch-s tokens
                a_in = a2aA_in if s == 0 else a2aB_in
                a_out = a2aA_out if s == 0 else a2aB_out
                nc.sync.dma_start(
                    out=a_in[:].rearrange("(blk p) t -> p blk t", p=128),
                    in_=ohT[:, s * N:(s + 1) * N].rearrange(
                        "p (blk t) -> p blk t", blk=NTB))
                nc.gpsimd.collective_compute(
                    "AllToAll", mybir.AluOpType.bypass, replica_groups=RG,
                    ins=[a_in[:]], outs=[a_out[:]])
                nc.sync.dma_start(
                    out=oh_all[:, :, s, :],
                    in_=a_out[:].rearrange("(blk p) t -> p blk t", p=128))

            def emit_proj(tcq):
                # own tokens tcq*128..: chunks 0-1 are batch-0 halves,
                # 2-3 batch-1 (oh_all free layout is [blk, s, 256])
                pp = psum_s.tile([128, 1024], f32, tag="s",
                                 name=f"pp_{tcq}")
                oh_flat = oh_all[:].rearrange("p blk s t -> p blk (s t)")
                for jh in range(C // 512):
                    for r in range(NCORES):
                        nc.tensor.matmul(
                            pp[:, jh * 512:(jh + 1) * 512],
                            oh_flat[:, r, tcq * 128:(tcq + 1) * 128],
                            w2_sb[:, r, jh * 512:(jh + 1) * 512],
                            start=(r == 0), stop=(r == NCORES - 1))
                ob = out_pool.tile([128, 1024], bf16, tag="ob",
                                   name=f"ob_{tcq}")
                nc.vector.tensor_copy(ob[:], pp[:])
                nc.sync.dma_start(
                    out=out_d[tcq * 128:(tcq + 1) * 128, :],
                    in_=ob[:])

            # ---- emission order: batch-0 qkv; batch-0 attention with
            # batch-1 qkv interleaved; w2 gather + batch-0 AllToAll and
            # half the projection during batch-1 attention ----
            for tb in range(NTB // B):
                emit_qkv(tb)
            for qb in range(NQB):
                emit_attention(0, qb)
                emit_qkv(NTB // B + qb)
            # w_proj gather: triggered here so ncfw runs it while the
            # engines chew on attention; needed only by the projection
            nc.gpsimd.collective_compute(
                "AllGather", mybir.AluOpType.bypass, replica_groups=RG,
                ins=[w2g_in[:]], outs=[w2g[:]])
            nc.sync.dma_start(
                out=w2_sb[:],
                in_=w2g[:].rearrange("(cc p) f -> p cc f", p=128))
            emit_a2a(0)
            for qb in range(NQB):
                emit_attention(1, qb)
                if qb >= 2:
                    emit_proj(qb - 2)  # batch-0 projection chunks 0-1
            emit_a2a(1)
            for tcq in range(2, 4):
                emit_proj(tcq)

    nc.compile()
    return nc


def get_program():
    if "nc" not in _CACHE:
        _CACHE["nc"] = _build_program()
    return _CACHE["nc"]


def build_null_program():
    """Tiny kernel for calibrating per-dispatch overhead in test harnesses."""
    import concourse.mybir as mybir
    import concourse.tile as tile
    from concourse import bacc

    f32 = mybir.dt.float32
    nc = bacc.Bacc("TRN2", target_bir_lowering=False, debug=False,
                   num_devices=NCORES)
    x_in = nc.dram_tensor("x", [128, 128], f32, kind="ExternalInput")
    y_out = nc.dram_tensor("y", [128, 128], f32, kind="ExternalOutput")
    with tile.TileContext(nc) as tc:
        with tc.tile_pool(name="p", bufs=1) as pool:
            t = pool.tile([128, 128], f32)
            nc.sync.dma_start(out=t[:], in_=x_in[:])
            nc.sync.dma_start(out=y_out[:], in_=t[:])
    nc.compile()
    x = np.zeros((128, 128), dtype=np.float32)
    return nc, [{"x": x} for _ in range(NCORES)]


def make_in_maps(x, w_qkv, b_qkv, w_proj):
    """Host-side sharding: per-core input dicts (bf16 weights/activations).

    Core c owns batch-0 tokens [c*256,(c+1)*256) and the same range of
    batch 1."""
    import ml_dtypes
    bf16 = ml_dtypes.bfloat16

    xbf = x.astype(bf16)
    ident = np.eye(128, dtype=bf16)

    def perm_shard(batch, core):
        # [128, cc*256+t] layout: row p holds x[batch, core*256+t,
        # cc*128+p] for cc-major contiguous per-partition DMA reads
        blk = xbf[batch, core * HS:(core + 1) * HS, :]  # [256, 1024]
        return np.ascontiguousarray(
            blk.T.reshape(CC_H, 128, HS).transpose(1, 0, 2).reshape(
                128, CC_H * HS))

    in_maps = []
    for core in range(NCORES):
        heads = [core * HPC + h for h in range(HPC)]
        cols = []
        for s in range(3):  # q, k, v groups
            for h in heads:
                cols.append(np.arange(s * C + h * HD, s * C + (h + 1) * HD))
        cols = np.concatenate(cols)
        w_loc = np.ascontiguousarray(w_qkv[:, cols].astype(bf16))
        b_loc = np.ascontiguousarray(
            b_qkv[cols].reshape(3, HPC * HD).T).astype(np.float32)
        rows = np.concatenate(
            [np.arange(h * HD, (h + 1) * HD) for h in heads])
        w2s = np.ascontiguousarray(w_proj[rows, :].astype(bf16))
        in_maps.append({
            "xTsA": perm_shard(0, core),
            "xTsB": perm_shard(1, core),
            "w_loc": w_loc,
            "b_loc": b_loc,
            "w2s": w2s,
            "ident": ident,
        })
    return in_maps


def combine_results(results, b_proj):
    """Host-side unshard: interleave the 8 token slices, add bias."""
    out = np.empty((B, N, C), dtype=np.float32)
    for core, res in enumerate(results):
        sl = np.asarray(res["out_s"], dtype=np.float32)
        out[0, core * HS:(core + 1) * HS] = sl[0:HS]
        out[1, core * HS:(core + 1) * HS] = sl[HS:2 * HS]
    out += b_proj.astype(np.float32)[None, None, :]
    return out


def kernel(x, w_qkv, b_qkv, w_proj, b_proj):
    from concourse.bass_utils import run_bass_kernel_spmd

    x = np.asarray(x, dtype=np.float32)
    w_qkv = np.asarray(w_qkv, dtype=np.float32)
    b_qkv = np.asarray(b_qkv, dtype=np.float32)
    w_proj = np.asarray(w_proj, dtype=np.float32)
    b_proj = np.asarray(b_proj, dtype=np.float32)

    nc = get_program()
    in_maps = make_in_maps(x, w_qkv, b_qkv, w_proj)
    res = run_bass_kernel_spmd(nc, in_maps, list(range(NCORES)))
    return combine_results(res.results, b_proj)
